# revision 12
# baseline (speedup 1.0000x reference)
"""Causal self-attention on 8 Trainium2 NeuronCores.

Sharding: batch (2) x head-groups (4 heads each) -> 8 cores.
Each core computes Q/K/V projections for its batch restricted to its 4
heads, causal softmax attention, and a partial output projection over its
head slice. The host sums the 4 partial projections per batch (+bo).

Device-side layouts are chosen so the PE never transposes the big S x S
attention matrix: scores are computed k-major (S^T[k,q]), the attention
weights are written to HBM k-major, and the output projection is produced
transposed ([D, S]); the host transposes both on assembly. Softmax sums
fall out of the A*V matmul through a ones-column appended to V. Strictly
upper-triangle blocks are never computed or written: the runtime pre-zeros
ExternalOutput buffers, which matches exp(-1e9-max) == 0 in the reference.

All matmuls run in bf16 with fp32 PSUM accumulation. Softmax skips the max
subtraction: scores here are O(1), and masked logits underflow exp() to an
exact 0 either way.
"""

import numpy as np
import ml_dtypes

B = 2
S = 2048
D = 1024
H = 16
DK = 64
HPC = 4          # heads per core
COLS = HPC * DK  # 256 projection columns per core
NCORES = 8
P = 128
NT = S // P      # 16 seq tiles / k-blocks
NG = S // 512    # 4 query groups of 512

_cached = None


def _build():
    import concourse.bacc as bacc
    import concourse.mybir as mybir
    from concourse.tile import TileContext

    f32 = mybir.dt.float32
    bf16 = mybir.dt.bfloat16
    Exp = mybir.ActivationFunctionType.Exp
    Ident = mybir.ActivationFunctionType.Identity

    nc = bacc.Bacc("TRN2", target_bir_lowering=False, debug=False,
                   num_devices=NCORES)

    xt_d = nc.dram_tensor("xt", [D, S], bf16, kind="ExternalInput")
    wq_d = nc.dram_tensor("wq", [D, COLS], bf16, kind="ExternalInput")
    wk_d = nc.dram_tensor("wk", [D, COLS], bf16, kind="ExternalInput")
    wv_d = nc.dram_tensor("wv", [D, COLS], bf16, kind="ExternalInput")
    wo_d = nc.dram_tensor("wo", [COLS, D], bf16, kind="ExternalInput")
    bq_d = nc.dram_tensor("bq", [COLS, 1], f32, kind="ExternalInput")
    bk_d = nc.dram_tensor("bk", [COLS, 1], f32, kind="ExternalInput")
    bv_d = nc.dram_tensor("bv", [P, COLS], f32, kind="ExternalInput")
    # attn_t[h, k, q] = attention_weights[h, q, k] (host transposes)
    attn_d = nc.dram_tensor("attn_t", [HPC, S, S], f32, kind="ExternalOutput")
    # pout_t[n, q] = partial output projection, transposed (host transposes)
    pout_d = nc.dram_tensor("pout_t", [D, S], f32, kind="ExternalOutput")

    KC = D // P  # 8 contraction chunks over d_model

    with TileContext(nc) as tc, (
        tc.tile_pool(name="persist", bufs=1)) as pers, (
        tc.tile_pool(name="psmm", space="PSUM", bufs=2)) as psmm, (
        tc.tile_pool(name="psav", space="PSUM", bufs=4)) as psav, (
        tc.tile_pool(name="work", bufs=2)) as work, (
        tc.tile_pool(name="atp", bufs=4)) as atp:

        def ptile(shape, dtype, name):
            return pers.tile(shape, dtype, tag=name, name=name)

        # mask_sb[:, j, :] zeroes exp(S^T) entries with q < k inside the
        # diagonal-containing 512-wide piece of k-block row j (j = kb % 4):
        # keep iff f - p - j*128 >= 0 (f = in-piece q, p = in-block k).
        mask_sb = ptile([P, 4, 512], bf16, name="mask_sb")
        nc.gpsimd.memset(mask_sb[:], 1.0)
        for j in range(4):
            nc.gpsimd.affine_select(
                out=mask_sb[:, j, :], in_=mask_sb[:, j, :],
                compare_op=mybir.AluOpType.is_ge, fill=0.0,
                base=-(j * P), pattern=[[1, 512]], channel_multiplier=-1,
            )

        # ---- inputs ----
        wq_sb = ptile([P, KC, COLS], bf16, name="wq_sb")
        nc.sync.dma_start(wq_sb[:], wq_d[:].rearrange("(c p) n -> p c n", p=P))
        wk_sb = ptile([P, KC, COLS], bf16, name="wk_sb")
        nc.sync.dma_start(wk_sb[:], wk_d[:].rearrange("(c p) n -> p c n", p=P))
        wv_sb = ptile([P, KC, COLS], bf16, name="wv_sb")
        nc.sync.dma_start(wv_sb[:], wv_d[:].rearrange("(c p) n -> p c n", p=P))
        wo_sb = ptile([P, 2, D], bf16, name="wo_sb")
        nc.sync.dma_start(wo_sb[:], wo_d[:].rearrange("(c p) n -> p c n", p=P))
        bq_sb = ptile([P, 2], f32, name="bq_sb")
        nc.sync.dma_start(bq_sb[:], bq_d[:].rearrange("(c p) o -> p (c o)", p=P))
        bk_sb = ptile([P, 2], f32, name="bk_sb")
        nc.sync.dma_start(bk_sb[:], bk_d[:].rearrange("(c p) o -> p (c o)", p=P))
        bv_sb = ptile([P, COLS], f32, name="bv_sb")
        nc.sync.dma_start(bv_sb[:], bv_d[:])
        # xt last, chunk-by-chunk, so the first projection matmuls can start
        # as soon as chunk 0 lands
        xt_sb = ptile([P, KC, S], bf16, name="xt_sb")
        for kc in range(KC):
            nc.sync.dma_start(
                xt_sb[:, kc, :], xt_d[kc * P:(kc + 1) * P, :])

        qt_sb = ptile([P, 2, S], bf16, name="qt_sb")
        kt_sb = ptile([P, 2, S], bf16, name="kt_sb")
        vaug_sb = ptile([P, NT, HPC * (DK + 1)], bf16, name="vaug_sb")
        ctxt_sb = ptile([P, 2, S], bf16, name="ctxt_sb")

        # ---- Q^T, K^T projections: out[m,n] = sum_d W[d,m] * xT[d,n] ----
        for w_sb, b_sb, out_sb in ((wq_sb, bq_sb, qt_sb),
                                   (wk_sb, bk_sb, kt_sb)):
            for m in range(2):
                for n in range(NG):
                    ps = psmm.tile([P, 1024], f32, tag="mm", name="ps_qk")
                    for kc in range(KC):
                        nc.tensor.matmul(
                            ps[:, 0:512],
                            lhsT=w_sb[:, kc, m * P:(m + 1) * P],
                            rhs=xt_sb[:, kc, n * 512:(n + 1) * 512],
                            start=(kc == 0), stop=(kc == KC - 1))
                    nc.scalar.activation(
                        out_sb[:, m, n * 512:(n + 1) * 512], ps[:, 0:512],
                        Ident, bias=b_sb[:, m:m + 1], scale=1.0)

        # ---- V (natural layout) + ones column for softmax sums ----
        nc.gpsimd.memset(vaug_sb[:], 1.0)
        for st in range(NT):
            ps = psmm.tile([P, 1024], f32, tag="mm", name="ps_v")
            for kc in range(KC):
                nc.tensor.matmul(
                    ps[:, 0:COLS],
                    lhsT=xt_sb[:, kc, st * P:(st + 1) * P],
                    rhs=wv_sb[:, kc, :],
                    start=(kc == 0), stop=(kc == KC - 1))
            nc.vector.tensor_add(
                out=vaug_sb[:, st, :].rearrange(
                    "p (h c) -> p h c", c=DK + 1)[:, :, 0:DK],
                in0=ps[:, 0:COLS].rearrange("p (h c) -> p h c", c=DK),
                in1=bv_sb[:].rearrange("p (h c) -> p h c", c=DK))

        # ---- attention per head (k-major throughout) ----
        deferred_flush = []

        def emit_flush(h, g, at, bv_t):
            # flush every (kb2, g) piece of A^T that was only waiting on
            # group g's denominators
            for kb2 in range(4 * g + 4):
                q0 = kb2 // 4 * 512
                # first piece of a k-block: leading q < k columns are
                # masked zeros; HBM is pre-zeroed, skip them
                lo = max(g * 512, kb2 * P)
                w = (g + 1) * 512 - lo
                stg = work.tile([P, 512], f32, tag="stage", bufs=10,
                                name="stg")
                eng = nc.gpsimd if ((kb2 + g) % 3 == 2) else nc.vector
                eng.tensor_mul(out=stg[:, 0:w],
                               in0=at[kb2][:, lo - q0:lo - q0 + w],
                               in1=bv_t[:, lo - g * 512:512])
                nc.sync.dma_start(
                    attn_d[h, kb2 * P:(kb2 + 1) * P, lo:lo + w],
                    stg[:, 0:w])

        for h in range(HPC):
            m = h // 2
            poff = (h % 2) * DK
            at = {}
            avp = [psav.tile([P, 512], f32, tag="av", name=f"ps_av{g}_{h}")
                   for g in range(NG)]

            def emit_scores(kb):
                g0 = kb // 4
                qstart = g0 * 512
                at[kb] = atp.tile([P, S - qstart], bf16,
                                  tag=f"at{g0}", bufs=(8, 6, 4, 4)[g0],
                                  name=f"at_{h}_{kb}")
                # scores^T + exp, two 512-groups per psum tile
                for gp in range(g0, NG, 2):
                    gn = min(2, NG - gp)
                    ps = psmm.tile([P, 1024], f32, tag="mm", name="ps_s")
                    for gi in range(gn):
                        g = gp + gi
                        nc.tensor.matmul(
                            ps[:, gi * 512:(gi + 1) * 512],
                            lhsT=kt_sb[poff:poff + DK, m, kb * P:(kb + 1) * P],
                            rhs=qt_sb[poff:poff + DK, m, g * 512:(g + 1) * 512],
                            start=True, stop=True)
                    o0 = gp * 512 - qstart
                    nc.scalar.activation(
                        at[kb][:, o0:o0 + gn * 512], ps[:, 0:gn * 512],
                        Exp, scale=0.125)
                nc.vector.tensor_mul(
                    out=at[kb][:, 0:512], in0=at[kb][:, 0:512],
                    in1=mask_sb[:, kb % 4, :])

            def emit_av(kb):
                # A^T V accumulation for every group this k-block feeds
                g0 = kb // 4
                for g in range(g0, NG):
                    qs = (g - g0) * 512
                    nc.tensor.matmul(
                        avp[g][0:DK + 1, :],
                        lhsT=vaug_sb[:, kb, h * (DK + 1):(h + 1) * (DK + 1)],
                        rhs=at[kb][:, qs:qs + 512],
                        start=(kb == 0), stop=(kb == 4 * g + 3))
                if kb % 4 != 3:
                    return
                # group g = (kb-3)/4 just finished: recip + ctx normalize
                g = kb // 4
                srow = work.tile([1, 512], f32, tag="srow", bufs=3,
                                 name="srow")
                nc.scalar.copy(srow[:], avp[g][DK:DK + 1, :])
                rrow = work.tile([1, 512], f32, tag="rrow", bufs=3,
                                 name="rrow")
                nc.vector.reciprocal_approx_fast(rrow[:], srow[:])
                bv_t = work.tile([P, 512], f32, tag="binv", bufs=5,
                                 name=f"binv_{h}_{g}")
                nc.gpsimd.partition_broadcast(bv_t[:], rrow[0:1, :])
                nc.vector.tensor_mul(
                    out=ctxt_sb[poff:poff + DK, m, g * 512:(g + 1) * 512],
                    in0=avp[g][0:DK, :], in1=bv_t[0:DK, :])
                emit_flush(h, g, at, bv_t)

            # software-pipelined: AV for k-block kb runs while the scores of
            # kb+1 stream, so the PE never stalls on the exp/mask round-trip
            for kb in range(NT):
                emit_scores(kb)
                if kb > 0:
                    emit_av(kb - 1)
            emit_av(NT - 1)

        # ---- partial output projection, transposed: pout^T[n, q] ----
        for mo in range(8):
            for qg in range(NG):
                pso = psmm.tile([P, 1024], f32, tag="mm", name="ps_o")
                for kc in range(2):
                    nc.tensor.matmul(
                        pso[:, 0:512],
                        lhsT=wo_sb[:, kc, mo * P:(mo + 1) * P],
                        rhs=ctxt_sb[:, kc, qg * 512:(qg + 1) * 512],
                        start=(kc == 0), stop=(kc == 1))
                post = work.tile([P, 512], f32, tag="stage", bufs=10,
                                 name="post")
                nc.scalar.copy(post[:], pso[:, 0:512])
                nc.sync.dma_start(
                    pout_d[mo * P:(mo + 1) * P, qg * 512:(qg + 1) * 512],
                    post[:])

        for args in deferred_flush:
            emit_flush(*args)

    nc.finalize()
    return nc


def _get_nc():
    global _cached
    if _cached is None:
        _cached = _build()
    return _cached


def kernel(x, Wq, bq, Wk, bk, Wv, bv, Wo, bo):
    from concourse.bass_utils import run_bass_kernel_spmd

    bf = ml_dtypes.bfloat16
    x = np.asarray(x, dtype=np.float32)
    Wq = np.asarray(Wq, dtype=np.float32)
    Wk = np.asarray(Wk, dtype=np.float32)
    Wv = np.asarray(Wv, dtype=np.float32)
    Wo = np.asarray(Wo, dtype=np.float32)
    bq = np.asarray(bq, dtype=np.float32)
    bk = np.asarray(bk, dtype=np.float32)
    bv = np.asarray(bv, dtype=np.float32)
    bo = np.asarray(bo, dtype=np.float32)

    xt = [np.ascontiguousarray(x[b].T).astype(bf) for b in range(B)]
    in_maps = []
    for c in range(NCORES):
        b = c // 4
        lo = (c % 4) * COLS
        hi = lo + COLS
        in_maps.append({
            "xt": xt[b],
            "wq": np.ascontiguousarray(Wq[:, lo:hi]).astype(bf),
            "wk": np.ascontiguousarray(Wk[:, lo:hi]).astype(bf),
            "wv": np.ascontiguousarray(Wv[:, lo:hi]).astype(bf),
            "wo": np.ascontiguousarray(Wo[lo:hi, :]).astype(bf),
            "bq": np.ascontiguousarray(bq[lo:hi].reshape(COLS, 1)),
            "bk": np.ascontiguousarray(bk[lo:hi].reshape(COLS, 1)),
            "bv": np.ascontiguousarray(
                np.broadcast_to(bv[lo:hi], (P, COLS))).astype(np.float32),
        })

    global _last_in_maps
    _last_in_maps = in_maps
    nc = _get_nc()
    res = run_bass_kernel_spmd(nc, in_maps, core_ids=list(range(NCORES)))

    out = np.empty((B, S, D), dtype=np.float32)
    attn = np.empty((B, H, S, S), dtype=np.float32)
    for b in range(B):
        acc = None
        for i in range(4):
            r = res.results[4 * b + i]
            pt = r["pout_t"]
            acc = pt if acc is None else acc + pt
            for hl in range(HPC):
                attn[b, i * HPC + hl] = r["attn_t"][hl].T
        out[b] = acc.T + bo
    return out, attn


# revision 13
# speedup vs baseline: 1.5831x; 1.5831x over previous
"""Causal self-attention on 8 Trainium2 NeuronCores.

Sharding: batch (2) x head-groups (4 heads each) -> 8 cores.
Each core computes Q/K/V projections for its batch restricted to its 4
heads, causal softmax attention, and a partial output projection over its
head slice. The host sums the 4 partial projections per batch (+bo).

Device-side layouts are chosen so the PE never transposes the big S x S
attention matrix: scores are computed k-major (S^T[k,q]), the attention
weights are written to HBM k-major, and the output projection is produced
transposed ([D, S]); the host transposes both on assembly. Softmax sums
fall out of the A*V matmul through a ones-column appended to V. Strictly
upper-triangle blocks are never computed or written: the runtime pre-zeros
ExternalOutput buffers, which matches exp(-1e9-max) == 0 in the reference.

All matmuls run in bf16 with fp32 PSUM accumulation. Softmax skips the max
subtraction: scores here are O(1), and masked logits underflow exp() to an
exact 0 either way.
"""

import numpy as np
import ml_dtypes

B = 2
S = 2048
D = 1024
H = 16
DK = 64
HPC = 4          # heads per core
COLS = HPC * DK  # 256 projection columns per core
NCORES = 8
P = 128
NT = S // P      # 16 seq tiles / k-blocks
NG = S // 512    # 4 query groups of 512

_cached = None


def _build():
    import concourse.bacc as bacc
    import concourse.mybir as mybir
    from concourse.tile import TileContext

    f32 = mybir.dt.float32
    bf16 = mybir.dt.bfloat16
    Exp = mybir.ActivationFunctionType.Exp
    Ident = mybir.ActivationFunctionType.Identity

    nc = bacc.Bacc("TRN2", target_bir_lowering=False, debug=False,
                   num_devices=NCORES)

    xt_d = nc.dram_tensor("xt", [D, S], bf16, kind="ExternalInput")
    wq_d = nc.dram_tensor("wq", [D, COLS], bf16, kind="ExternalInput")
    wk_d = nc.dram_tensor("wk", [D, COLS], bf16, kind="ExternalInput")
    wv_d = nc.dram_tensor("wv", [D, COLS], bf16, kind="ExternalInput")
    wo_d = nc.dram_tensor("wo", [COLS, D], bf16, kind="ExternalInput")
    bq_d = nc.dram_tensor("bq", [COLS, 1], f32, kind="ExternalInput")
    bk_d = nc.dram_tensor("bk", [COLS, 1], f32, kind="ExternalInput")
    bv_d = nc.dram_tensor("bv", [P, COLS], f32, kind="ExternalInput")
    # attn_t[h, k, q] = unnormalized exp(scores)[h, q, k] in bf16; the host
    # transposes and multiplies by the softmax reciprocals (sums output)
    attn_d = nc.dram_tensor("attn_t", [HPC, S, S], bf16, kind="ExternalOutput")
    sums_d = nc.dram_tensor("sums", [HPC, S], f32, kind="ExternalOutput")
    # pout_t[n, q] = partial output projection, transposed (host transposes)
    pout_d = nc.dram_tensor("pout_t", [D, S], f32, kind="ExternalOutput")

    KC = D // P  # 8 contraction chunks over d_model

    with TileContext(nc) as tc, (
        tc.tile_pool(name="persist", bufs=1)) as pers, (
        tc.tile_pool(name="psmm", space="PSUM", bufs=2)) as psmm, (
        tc.tile_pool(name="psav", space="PSUM", bufs=4)) as psav, (
        tc.tile_pool(name="work", bufs=2)) as work, (
        tc.tile_pool(name="atp", bufs=4)) as atp:

        def ptile(shape, dtype, name):
            return pers.tile(shape, dtype, tag=name, name=name)

        # mask_sb[:, j, :] zeroes exp(S^T) entries with q < k inside the
        # diagonal-containing 512-wide piece of k-block row j (j = kb % 4):
        # keep iff f - p - j*128 >= 0 (f = in-piece q, p = in-block k).
        mask_sb = ptile([P, 4, 512], bf16, name="mask_sb")
        nc.gpsimd.memset(mask_sb[:], 1.0)
        for j in range(4):
            nc.gpsimd.affine_select(
                out=mask_sb[:, j, :], in_=mask_sb[:, j, :],
                compare_op=mybir.AluOpType.is_ge, fill=0.0,
                base=-(j * P), pattern=[[1, 512]], channel_multiplier=-1,
            )

        # ---- inputs ----
        wq_sb = ptile([P, KC, COLS], bf16, name="wq_sb")
        nc.sync.dma_start(wq_sb[:], wq_d[:].rearrange("(c p) n -> p c n", p=P))
        wk_sb = ptile([P, KC, COLS], bf16, name="wk_sb")
        nc.sync.dma_start(wk_sb[:], wk_d[:].rearrange("(c p) n -> p c n", p=P))
        wv_sb = ptile([P, KC, COLS], bf16, name="wv_sb")
        nc.sync.dma_start(wv_sb[:], wv_d[:].rearrange("(c p) n -> p c n", p=P))
        wo_sb = ptile([P, 2, D], bf16, name="wo_sb")
        nc.sync.dma_start(wo_sb[:], wo_d[:].rearrange("(c p) n -> p c n", p=P))
        bq_sb = ptile([P, 2], f32, name="bq_sb")
        nc.sync.dma_start(bq_sb[:], bq_d[:].rearrange("(c p) o -> p (c o)", p=P))
        bk_sb = ptile([P, 2], f32, name="bk_sb")
        nc.sync.dma_start(bk_sb[:], bk_d[:].rearrange("(c p) o -> p (c o)", p=P))
        bv_sb = ptile([P, COLS], f32, name="bv_sb")
        nc.sync.dma_start(bv_sb[:], bv_d[:])
        # xt last, chunk-by-chunk, so the first projection matmuls can start
        # as soon as chunk 0 lands
        xt_sb = ptile([P, KC, S], bf16, name="xt_sb")
        for kc in range(KC):
            nc.sync.dma_start(
                xt_sb[:, kc, :], xt_d[kc * P:(kc + 1) * P, :])

        qt_sb = ptile([P, 2, S], bf16, name="qt_sb")
        kt_sb = ptile([P, 2, S], bf16, name="kt_sb")
        vaug_sb = ptile([P, NT, HPC * (DK + 1)], bf16, name="vaug_sb")
        ctxt_sb = ptile([P, 2, S], bf16, name="ctxt_sb")

        # ---- Q^T, K^T projections: out[m,n] = sum_d W[d,m] * xT[d,n] ----
        for w_sb, b_sb, out_sb in ((wq_sb, bq_sb, qt_sb),
                                   (wk_sb, bk_sb, kt_sb)):
            for m in range(2):
                for n in range(NG):
                    ps = psmm.tile([P, 1024], f32, tag="mm", name="ps_qk")
                    for kc in range(KC):
                        nc.tensor.matmul(
                            ps[:, 0:512],
                            lhsT=w_sb[:, kc, m * P:(m + 1) * P],
                            rhs=xt_sb[:, kc, n * 512:(n + 1) * 512],
                            start=(kc == 0), stop=(kc == KC - 1))
                    nc.scalar.activation(
                        out_sb[:, m, n * 512:(n + 1) * 512], ps[:, 0:512],
                        Ident, bias=b_sb[:, m:m + 1], scale=1.0)

        # ---- V (natural layout) + ones column for softmax sums ----
        nc.gpsimd.memset(vaug_sb[:], 1.0)
        for st in range(NT):
            ps = psmm.tile([P, 1024], f32, tag="mm", name="ps_v")
            for kc in range(KC):
                nc.tensor.matmul(
                    ps[:, 0:COLS],
                    lhsT=xt_sb[:, kc, st * P:(st + 1) * P],
                    rhs=wv_sb[:, kc, :],
                    start=(kc == 0), stop=(kc == KC - 1))
            nc.vector.tensor_add(
                out=vaug_sb[:, st, :].rearrange(
                    "p (h c) -> p h c", c=DK + 1)[:, :, 0:DK],
                in0=ps[:, 0:COLS].rearrange("p (h c) -> p h c", c=DK),
                in1=bv_sb[:].rearrange("p (h c) -> p h c", c=DK))

        # ---- attention per head (k-major throughout) ----
        for h in range(HPC):
            m = h // 2
            poff = (h % 2) * DK
            at = {}
            avp = [psav.tile([P, 512], f32, tag="av", name=f"ps_av{g}_{h}")
                   for g in range(NG)]

            def emit_scores(kb):
                g0 = kb // 4
                qstart = g0 * 512
                at[kb] = atp.tile([P, S - qstart], bf16,
                                  tag=f"at{g0}", bufs=(8, 6, 4, 4)[g0],
                                  name=f"at_{h}_{kb}")
                # scores^T + exp, two 512-groups per psum tile
                for gp in range(g0, NG, 2):
                    gn = min(2, NG - gp)
                    ps = psmm.tile([P, 1024], f32, tag="mm", name="ps_s")
                    for gi in range(gn):
                        g = gp + gi
                        nc.tensor.matmul(
                            ps[:, gi * 512:(gi + 1) * 512],
                            lhsT=kt_sb[poff:poff + DK, m, kb * P:(kb + 1) * P],
                            rhs=qt_sb[poff:poff + DK, m, g * 512:(g + 1) * 512],
                            start=True, stop=True)
                    o0 = gp * 512 - qstart
                    nc.scalar.activation(
                        at[kb][:, o0:o0 + gn * 512], ps[:, 0:gn * 512],
                        Exp, scale=0.125)
                nc.vector.tensor_mul(
                    out=at[kb][:, 0:512], in0=at[kb][:, 0:512],
                    in1=mask_sb[:, kb % 4, :])
                nc.sync.dma_start(
                    attn_d[h, kb * P:(kb + 1) * P, qstart:S], at[kb][:])

            def emit_av(kb):
                # A^T V accumulation for every group this k-block feeds
                g0 = kb // 4
                for g in range(g0, NG):
                    qs = (g - g0) * 512
                    nc.tensor.matmul(
                        avp[g][0:DK + 1, :],
                        lhsT=vaug_sb[:, kb, h * (DK + 1):(h + 1) * (DK + 1)],
                        rhs=at[kb][:, qs:qs + 512],
                        start=(kb == 0), stop=(kb == 4 * g + 3))
                if kb % 4 != 3:
                    return
                # group g = (kb-3)/4 just finished: export the softmax
                # sums for the host-side normalize, and normalize ctx here
                g = kb // 4
                srow = work.tile([1, 512], f32, tag="srow", bufs=3,
                                 name="srow")
                nc.scalar.copy(srow[:], avp[g][DK:DK + 1, :])
                nc.sync.dma_start(
                    sums_d[h:h + 1, g * 512:(g + 1) * 512], srow[:])
                rrow = work.tile([1, 512], f32, tag="rrow", bufs=3,
                                 name="rrow")
                nc.vector.reciprocal_approx_fast(rrow[:], srow[:])
                bv_t = work.tile([DK, 512], f32, tag="binv", bufs=4,
                                 name=f"binv_{h}_{g}")
                nc.gpsimd.partition_broadcast(bv_t[:], rrow[0:1, :])
                nc.vector.tensor_mul(
                    out=ctxt_sb[poff:poff + DK, m, g * 512:(g + 1) * 512],
                    in0=avp[g][0:DK, :], in1=bv_t[:])

            # software-pipelined: AV for k-block kb runs while the scores of
            # kb+1 stream, so the PE never stalls on the exp/mask round-trip
            for kb in range(NT):
                emit_scores(kb)
                if kb > 0:
                    emit_av(kb - 1)
            emit_av(NT - 1)

        # ---- partial output projection, transposed: pout^T[n, q] ----
        for mo in range(8):
            for qg in range(NG):
                pso = psmm.tile([P, 1024], f32, tag="mm", name="ps_o")
                for kc in range(2):
                    nc.tensor.matmul(
                        pso[:, 0:512],
                        lhsT=wo_sb[:, kc, mo * P:(mo + 1) * P],
                        rhs=ctxt_sb[:, kc, qg * 512:(qg + 1) * 512],
                        start=(kc == 0), stop=(kc == 1))
                post = work.tile([P, 512], f32, tag="post", bufs=6,
                                 name="post")
                nc.vector.tensor_copy(post[:], pso[:, 0:512])
                nc.sync.dma_start(
                    pout_d[mo * P:(mo + 1) * P, qg * 512:(qg + 1) * 512],
                    post[:])

    nc.finalize()
    return nc


def _get_nc():
    global _cached
    if _cached is None:
        _cached = _build()
    return _cached


def kernel(x, Wq, bq, Wk, bk, Wv, bv, Wo, bo):
    from concourse.bass_utils import run_bass_kernel_spmd

    bf = ml_dtypes.bfloat16
    x = np.asarray(x, dtype=np.float32)
    Wq = np.asarray(Wq, dtype=np.float32)
    Wk = np.asarray(Wk, dtype=np.float32)
    Wv = np.asarray(Wv, dtype=np.float32)
    Wo = np.asarray(Wo, dtype=np.float32)
    bq = np.asarray(bq, dtype=np.float32)
    bk = np.asarray(bk, dtype=np.float32)
    bv = np.asarray(bv, dtype=np.float32)
    bo = np.asarray(bo, dtype=np.float32)

    xt = [np.ascontiguousarray(x[b].T).astype(bf) for b in range(B)]
    in_maps = []
    for c in range(NCORES):
        b = c // 4
        lo = (c % 4) * COLS
        hi = lo + COLS
        in_maps.append({
            "xt": xt[b],
            "wq": np.ascontiguousarray(Wq[:, lo:hi]).astype(bf),
            "wk": np.ascontiguousarray(Wk[:, lo:hi]).astype(bf),
            "wv": np.ascontiguousarray(Wv[:, lo:hi]).astype(bf),
            "wo": np.ascontiguousarray(Wo[lo:hi, :]).astype(bf),
            "bq": np.ascontiguousarray(bq[lo:hi].reshape(COLS, 1)),
            "bk": np.ascontiguousarray(bk[lo:hi].reshape(COLS, 1)),
            "bv": np.ascontiguousarray(
                np.broadcast_to(bv[lo:hi], (P, COLS))).astype(np.float32),
        })

    global _last_in_maps
    _last_in_maps = in_maps
    nc = _get_nc()
    res = run_bass_kernel_spmd(nc, in_maps, core_ids=list(range(NCORES)))

    out = np.empty((B, S, D), dtype=np.float32)
    attn = np.empty((B, H, S, S), dtype=np.float32)
    for b in range(B):
        acc = None
        for i in range(4):
            r = res.results[4 * b + i]
            pt = r["pout_t"]
            acc = pt if acc is None else acc + pt
            ex = r["attn_t"]          # bf16 [HPC, S, S], k-major, unnormalized
            recips = 1.0 / r["sums"]  # [HPC, S] f32
            for hl in range(HPC):
                np.multiply(ex[hl].T.astype(np.float32),
                            recips[hl][:, None],
                            out=attn[b, i * HPC + hl])
        out[b] = acc.T + bo
    return out, attn
